# revision 4
# baseline (speedup 1.0000x reference)
"""AttentionPooling (segment softmax + weighted segment sum) on 8 trn2 cores.

Strategy: shard whole segments across cores (sorted batch -> contiguous node
ranges).  The host pre-casts x to bf16 and ships it twice: channel-partitioned
(xt, for the MLP matmuls) and node-partitioned with an appended ones-channel
(xb, for the weighted-sum matmul whose 257th output column is the softmax
denominator).  Total HBM traffic per core = 2 x 32 MB bf16 = the same 64 MB a
single fp32 copy would cost, with zero on-device casts/transposes/bounces.
Per 2048-node chunk: 8 MLP matmuls (512-col), 16 1-col score matmuls, tanh+exp
on ACT, bf16 compare/mult on DVE for we = onehot(segment)*e, 16 wsum matmuls
accumulating (64,257) in PSUM.  Softmax max-subtraction is skipped:
|s| <= ||W2||_1 + |b2| ~ 28, exp stays in fp32/bf16 range.
"""

from contextlib import ExitStack

import numpy as np
import ml_dtypes

import concourse.bass as bass
import concourse.bacc as bacc
import concourse.tile as tile
from concourse import mybir
from concourse.bass_utils import run_bass_kernel_spmd

N_CORES = 8
NUM_GRAPHS = 512
SEGS_PER_CORE = NUM_GRAPHS // N_CORES  # 64
D = 256          # in channels
H = 128          # hidden
P = 128          # partitions
TILE_N = 128     # nodes per score tile
CHUNK_T = 16     # tiles per chunk
CHUNK_N = TILE_N * CHUNK_T  # 2048 nodes per chunk

_BF16 = mybir.dt.bfloat16
_F32 = mybir.dt.float32


def _build_program(n_chunks: int, b2_val: float):
    nc = bacc.Bacc()
    NC = n_chunks

    xt_d = nc.declare_dram_parameter("xt", [NC, P, 2, CHUNK_N], _BF16, isOutput=False)
    xb_d = nc.declare_dram_parameter("xb", [NC, P, CHUNK_T, D + 1], _BF16, isOutput=False)
    bt_d = nc.declare_dram_parameter("bt", [P, NC, CHUNK_T], _BF16, isOutput=False)
    io_d = nc.declare_dram_parameter("iota", [P, CHUNK_T * SEGS_PER_CORE], _BF16, isOutput=False)
    w1_d = nc.declare_dram_parameter("w1", [P, 2, H], _BF16, isOutput=False)
    w2_d = nc.declare_dram_parameter("w2", [P, 1], _BF16, isOutput=False)
    b1_d = nc.declare_dram_parameter("b1", [P, 1], _F32, isOutput=False)
    out_d = nc.declare_dram_parameter("out_g", [SEGS_PER_CORE, D], _F32, isOutput=True)

    with tile.TileContext(nc) as tc, ExitStack() as ctx:
        const_pool = ctx.enter_context(tc.tile_pool(name="consts", bufs=1))
        xt_pool = ctx.enter_context(tc.tile_pool(name="xt", bufs=4))
        xb_pool = ctx.enter_context(tc.tile_pool(name="xb", bufs=4))
        h_pool = ctx.enter_context(tc.tile_pool(name="h", bufs=2))
        we_pool = ctx.enter_context(tc.tile_pool(name="we", bufs=3))
        e_pool = ctx.enter_context(tc.tile_pool(name="e", bufs=2))
        fin_pool = ctx.enter_context(tc.tile_pool(name="fin", bufs=1))
        psum_h = ctx.enter_context(
            tc.tile_pool(name="psum_h", bufs=3, space=bass.MemorySpace.PSUM))
        psum_s = ctx.enter_context(
            tc.tile_pool(name="psum_s", bufs=2, space=bass.MemorySpace.PSUM))
        psum_acc = ctx.enter_context(
            tc.tile_pool(name="psum_acc", bufs=1, space=bass.MemorySpace.PSUM))

        # ---- constants / weights ----
        w1_sb = const_pool.tile([P, 2, H], _BF16, tag="w1")
        nc.scalar.dma_start(w1_sb[:], w1_d[:])
        w2_sb = const_pool.tile([P, 1], _BF16, tag="w2")
        nc.scalar.dma_start(w2_sb[:], w2_d[:])
        b1_sb = const_pool.tile([P, 1], _F32, tag="b1")
        nc.scalar.dma_start(b1_sb[:], b1_d[:])
        bt_sb = const_pool.tile([P, NC, CHUNK_T], _BF16, tag="bt")
        nc.scalar.dma_start(bt_sb[:], bt_d[:])
        io_sb = const_pool.tile([P, CHUNK_T * SEGS_PER_CORE], _BF16, tag="iota")
        nc.scalar.dma_start(io_sb[:], io_d[:])
        io_v = io_sb[:].rearrange("p (t g) -> p t g", g=SEGS_PER_CORE)

        acc_ps = psum_acc.tile([SEGS_PER_CORE, D + 1], _F32, tag="acc")

        saved = {}

        def wsum_tile(c, t):
            we, xb = saved[c]
            nc.tensor.matmul(acc_ps[:], we[:, t, :], xb[:, t, :],
                             start=(c == 0 and t == 0),
                             stop=(c == n_chunks - 1 and t == CHUNK_T - 1),
                             skip_group_check=True)
            if t == CHUNK_T - 1:
                del saved[c]

        def emit_chunk(c):
            # interleaves chunk c's score pipeline with chunk c-1's wsum
            # matmuls so the PE stream alternates PSUM banks and stays dense.
            if c < n_chunks:
                xt = xt_pool.tile([P, 2, CHUNK_N], _BF16, tag="xt")
                nc.sync.dma_start(xt[:], xt_d[c])
                xb = xb_pool.tile([P, CHUNK_T, D + 1], _BF16, tag="xb")
                nc.gpsimd.dma_start(xb[:], xb_d[c])

                h_bf = h_pool.tile([P, CHUNK_N], _BF16, tag="h")
                ps_s = psum_s.tile([P, CHUNK_T], _F32, tag="ps_s")
                for s in range(CHUNK_N // 512):
                    ph = psum_h.tile([P, 512], _F32, tag="ph")
                    sl = slice(s * 512, (s + 1) * 512)
                    nc.tensor.matmul(ph[:], w1_sb[:, 0, :], xt[:, 0, sl],
                                     start=True, stop=False)
                    nc.tensor.matmul(ph[:], w1_sb[:, 1, :], xt[:, 1, sl],
                                     start=False, stop=True)
                    nc.scalar.activation(h_bf[:, sl], ph[:],
                                         mybir.ActivationFunctionType.Tanh,
                                         bias=b1_sb[:])
                    for t in range(4 * s, 4 * s + 4):
                        nc.tensor.matmul(ps_s[:, t:t + 1],
                                         h_bf[:, t * TILE_N:(t + 1) * TILE_N],
                                         w2_sb[:], start=True, stop=True)
                        if c >= 1:
                            wsum_tile(c - 1, t)

                e_col = e_pool.tile([P, CHUNK_T], _BF16, tag="e")
                nc.scalar.activation(e_col[:], ps_s[:],
                                     mybir.ActivationFunctionType.Exp,
                                     bias=float(b2_val))

                # we[p, t, g] = (bt[p, t] == g) * e[p, t]   (all bf16)
                we = we_pool.tile([P, CHUNK_T, SEGS_PER_CORE], _BF16, tag="we")
                nc.vector.tensor_tensor(
                    we[:], io_v,
                    bt_sb[:, c, :].unsqueeze(2).broadcast_to(
                        [P, CHUNK_T, SEGS_PER_CORE]),
                    mybir.AluOpType.is_equal)
                nc.vector.tensor_tensor(
                    we[:], we[:],
                    e_col[:].unsqueeze(2).broadcast_to(
                        [P, CHUNK_T, SEGS_PER_CORE]),
                    mybir.AluOpType.mult)
                saved[c] = (we, xb)
            else:
                for t in range(CHUNK_T):
                    wsum_tile(c - 1, t)

        for c in range(n_chunks + 1):
            emit_chunk(c)

        # ---- epilogue: out = acc[:, :256] / acc[:, 256] ----
        den_sb = fin_pool.tile([SEGS_PER_CORE, 1], _F32, tag="den")
        nc.vector.tensor_scalar_add(den_sb[:], acc_ps[:, D:D + 1], 1e-30)
        rec_sb = fin_pool.tile([SEGS_PER_CORE, 1], _F32, tag="rec")
        nc.vector.reciprocal(rec_sb[:], den_sb[:])
        out_sb = fin_pool.tile([SEGS_PER_CORE, D], _F32, tag="out")
        nc.vector.tensor_scalar_mul(out_sb[:], acc_ps[:, 0:D], rec_sb[:])
        nc.sync.dma_start(out_d[:], out_sb[:])

    return nc


def _prepare_inputs(x, W1, b1, W2, b2, batch):
    x = np.ascontiguousarray(np.asarray(x, np.float32))
    batch = np.asarray(batch).astype(np.int64)
    # core k owns segments [64k, 64(k+1)); sorted batch -> contiguous ranges
    bounds = np.searchsorted(batch, np.arange(0, NUM_GRAPHS + 1, SEGS_PER_CORE))
    counts = np.diff(bounds)
    nmax = int(np.max(counts))
    n_chunks = max(1, -(-nmax // CHUNK_N))
    nmax_pad = n_chunks * CHUNK_N
    NC = n_chunks
    bf16 = ml_dtypes.bfloat16

    w1_h = np.ascontiguousarray(
        np.asarray(W1, np.float32).reshape(2, P, H).transpose(1, 0, 2)).astype(bf16)
    w2_h = np.asarray(W2, np.float32).reshape(H, 1).astype(bf16)
    b1_h = np.asarray(b1, np.float32).reshape(H, 1)
    iota_h = np.ascontiguousarray(np.broadcast_to(
        np.tile(np.arange(SEGS_PER_CORE, dtype=np.float32), CHUNK_T),
        (P, CHUNK_T * SEGS_PER_CORE))).astype(bf16)

    in_maps = []
    for k in range(N_CORES):
        lo, hi = int(bounds[k]), int(bounds[k + 1])
        cnt = hi - lo
        xcb = np.zeros((nmax_pad, D), bf16)
        xcb[:cnt] = x[lo:hi].astype(bf16)
        # xt[c, p, hf, n] = x[c*2048 + n, 128*hf + p]  (channel-partitioned)
        xt = np.ascontiguousarray(
            xcb.reshape(NC, CHUNK_N, 2, P).transpose(0, 3, 2, 1))
        # xb[c, p, t, ch] = x[c*2048 + 128*t + p, ch]; xb[..., 256] = 1
        xb = np.empty((NC, P, CHUNK_T, D + 1), bf16)
        xb[..., :D] = xcb.reshape(NC, CHUNK_T, P, D).transpose(0, 2, 1, 3)
        xb[..., D] = 1.0
        # bt[p, c, t] = local segment of node c*2048 + 128*t + p  (-1 = pad)
        lb = np.full((nmax_pad,), -1.0, np.float32)
        lb[:cnt] = (batch[lo:hi] - k * SEGS_PER_CORE).astype(np.float32)
        bt = np.ascontiguousarray(
            lb.reshape(NC, CHUNK_T, P).transpose(2, 0, 1)).astype(bf16)
        in_maps.append({
            "xt": xt,
            "xb": xb,
            "bt": bt,
            "iota": iota_h,
            "w1": w1_h,
            "w2": w2_h,
            "b1": b1_h,
        })
    return in_maps, n_chunks


def run(x, W1, b1, W2, b2, batch, trace=False, trace_kwargs=None):
    in_maps, n_chunks = _prepare_inputs(x, W1, b1, W2, b2, batch)
    nc = _build_program(n_chunks, float(np.asarray(b2).reshape(-1)[0]))
    nc.finalize()
    res = run_bass_kernel_spmd(nc, in_maps, list(range(N_CORES)),
                               trace=trace, **(trace_kwargs or {}))
    out = np.concatenate([np.asarray(res.results[k]["out_g"], np.float32)
                          for k in range(N_CORES)], axis=0)
    return out, res


def kernel(x, W1, b1, W2, b2, batch):
    out, _ = run(x, W1, b1, W2, b2, batch)
    return out


# revision 5
# speedup vs baseline: 1.0023x; 1.0023x over previous
"""AttentionPooling (segment softmax + weighted segment sum) on 8 trn2 cores.

Strategy: shard whole segments across cores (sorted batch -> contiguous node
ranges).  The host pre-casts x to bf16 and ships it twice: channel-partitioned
(xt, for the MLP matmuls) and node-partitioned with an appended ones-channel
(xb, for the weighted-sum matmul whose 257th output column is the softmax
denominator).  Total HBM traffic per core = 2 x 32 MB bf16 = the same 64 MB a
single fp32 copy would cost, with zero on-device casts/transposes/bounces.
Per 2048-node chunk: 8 MLP matmuls (512-col), 16 1-col score matmuls, tanh+exp
on ACT, bf16 compare/mult on DVE for we = onehot(segment)*e, 16 wsum matmuls
accumulating (64,257) in PSUM.  Softmax max-subtraction is skipped:
|s| <= ||W2||_1 + |b2| ~ 28, exp stays in fp32/bf16 range.
"""

from contextlib import ExitStack

import numpy as np
import ml_dtypes

import concourse.bass as bass
import concourse.bacc as bacc
import concourse.tile as tile
from concourse import mybir
from concourse.bass_utils import run_bass_kernel_spmd

N_CORES = 8
NUM_GRAPHS = 512
SEGS_PER_CORE = NUM_GRAPHS // N_CORES  # 64
D = 256          # in channels
H = 128          # hidden
P = 128          # partitions
TILE_N = 128     # nodes per score tile
CHUNK_T = 16     # tiles per chunk
CHUNK_N = TILE_N * CHUNK_T  # 2048 nodes per chunk

_BF16 = mybir.dt.bfloat16
_F32 = mybir.dt.float32


def _build_program(n_chunks: int, b2_val: float):
    nc = bacc.Bacc()
    NC = n_chunks

    xt_d = nc.declare_dram_parameter("xt", [NC, P, 2, CHUNK_N], _BF16, isOutput=False)
    xb_d = nc.declare_dram_parameter("xb", [NC, P, CHUNK_T, D + 1], _BF16, isOutput=False)
    bt_d = nc.declare_dram_parameter("bt", [P, NC, CHUNK_T], _BF16, isOutput=False)
    io_d = nc.declare_dram_parameter("iota", [P, CHUNK_T * SEGS_PER_CORE], _BF16, isOutput=False)
    w1_d = nc.declare_dram_parameter("w1", [P, 2, H], _BF16, isOutput=False)
    w2_d = nc.declare_dram_parameter("w2", [P, 1], _BF16, isOutput=False)
    b1_d = nc.declare_dram_parameter("b1", [P, 1], _F32, isOutput=False)
    out_d = nc.declare_dram_parameter("out_g", [SEGS_PER_CORE, D], _F32, isOutput=True)

    with tile.TileContext(nc) as tc, ExitStack() as ctx:
        const_pool = ctx.enter_context(tc.tile_pool(name="consts", bufs=1))
        xt_pool = ctx.enter_context(tc.tile_pool(name="xt", bufs=4))
        xb_pool = ctx.enter_context(tc.tile_pool(name="xb", bufs=4))
        h_pool = ctx.enter_context(tc.tile_pool(name="h", bufs=2))
        we_pool = ctx.enter_context(tc.tile_pool(name="we", bufs=3))
        e_pool = ctx.enter_context(tc.tile_pool(name="e", bufs=2))
        fin_pool = ctx.enter_context(tc.tile_pool(name="fin", bufs=1))
        psum_h = ctx.enter_context(
            tc.tile_pool(name="psum_h", bufs=3, space=bass.MemorySpace.PSUM))
        psum_s = ctx.enter_context(
            tc.tile_pool(name="psum_s", bufs=2, space=bass.MemorySpace.PSUM))
        psum_acc = ctx.enter_context(
            tc.tile_pool(name="psum_acc", bufs=1, space=bass.MemorySpace.PSUM))

        # ---- constants / weights ----
        w1_sb = const_pool.tile([P, 2, H], _BF16, tag="w1")
        nc.scalar.dma_start(w1_sb[:], w1_d[:])
        w2_sb = const_pool.tile([P, 1], _BF16, tag="w2")
        nc.scalar.dma_start(w2_sb[:], w2_d[:])
        b1_sb = const_pool.tile([P, 1], _F32, tag="b1")
        nc.scalar.dma_start(b1_sb[:], b1_d[:])
        bt_sb = const_pool.tile([P, NC, CHUNK_T], _BF16, tag="bt")
        nc.scalar.dma_start(bt_sb[:], bt_d[:])
        io_sb = const_pool.tile([P, CHUNK_T * SEGS_PER_CORE], _BF16, tag="iota")
        nc.scalar.dma_start(io_sb[:], io_d[:])
        io_v = io_sb[:].rearrange("p (t g) -> p t g", g=SEGS_PER_CORE)

        acc_ps = psum_acc.tile([SEGS_PER_CORE, D + 1], _F32, tag="acc")

        # software pipeline over global slices k = 4c + s.  At step k we emit:
        #   1. the MLP matmul pair + tanh for slice k,
        #   2. the score matmuls for slice k-1 (so the PE never waits on the
        #      tanh latency of its own slice),
        #   3. one 4-tile wsum group, delayed 5 slices behind its chunk's
        #      scores (so `we` is long since ready).
        state = {}

        def mlp_slice(k):
            c, s = divmod(k, 4)
            if s == 0:
                xt = xt_pool.tile([P, 2, CHUNK_N], _BF16, tag="xt")
                nc.sync.dma_start(xt[:], xt_d[c])
                xb = xb_pool.tile([P, CHUNK_T, D + 1], _BF16, tag="xb")
                nc.gpsimd.dma_start(xb[:], xb_d[c])
                h_bf = h_pool.tile([P, CHUNK_N], _BF16, tag="h")
                ps_s = psum_s.tile([P, CHUNK_T], _F32, tag="ps_s")
                state[c] = {"xt": xt, "xb": xb, "h": h_bf, "ps": ps_s}
            st = state[c]
            ph = psum_h.tile([P, 512], _F32, tag="ph")
            sl = slice(s * 512, (s + 1) * 512)
            nc.tensor.matmul(ph[:], w1_sb[:, 0, :], st["xt"][:, 0, sl],
                             start=True, stop=False)
            nc.tensor.matmul(ph[:], w1_sb[:, 1, :], st["xt"][:, 1, sl],
                             start=False, stop=True)
            nc.scalar.activation(st["h"][:, sl], ph[:],
                                 mybir.ActivationFunctionType.Tanh,
                                 bias=b1_sb[:])

        def score_slice(k):
            c, s = divmod(k, 4)
            st = state[c]
            for t in range(4 * s, 4 * s + 4):
                nc.tensor.matmul(st["ps"][:, t:t + 1],
                                 st["h"][:, t * TILE_N:(t + 1) * TILE_N],
                                 w2_sb[:], start=True, stop=True)
            if s == 3:
                e_col = e_pool.tile([P, CHUNK_T], _BF16, tag="e")
                nc.scalar.activation(e_col[:], st["ps"][:],
                                     mybir.ActivationFunctionType.Exp,
                                     bias=float(b2_val))
                # we[p, t, g] = (bt[p, t] == g) * e[p, t]   (all bf16)
                we = we_pool.tile([P, CHUNK_T, SEGS_PER_CORE], _BF16, tag="we")
                nc.vector.tensor_tensor(
                    we[:], io_v,
                    bt_sb[:, c, :].unsqueeze(2).broadcast_to(
                        [P, CHUNK_T, SEGS_PER_CORE]),
                    mybir.AluOpType.is_equal)
                nc.vector.tensor_tensor(
                    we[:], we[:],
                    e_col[:].unsqueeze(2).broadcast_to(
                        [P, CHUNK_T, SEGS_PER_CORE]),
                    mybir.AluOpType.mult)
                st["we"] = we

        def wsum_group(j):
            c, grp = divmod(j, 4)
            st = state[c]
            for t in range(4 * grp, 4 * grp + 4):
                nc.tensor.matmul(acc_ps[:], st["we"][:, t, :],
                                 st["xb"][:, t, :],
                                 start=(c == 0 and t == 0),
                                 stop=(c == n_chunks - 1 and t == CHUNK_T - 1),
                                 skip_group_check=True)
            if grp == 3:
                del state[c]

        n_slices = 4 * n_chunks
        for k in range(n_slices + 14):
            if k < n_slices:
                mlp_slice(k)
            if 1 <= k <= n_slices:
                score_slice(k - 1)
            if 9 <= k < n_slices + 9:
                wsum_group(k - 9)

        # ---- epilogue: out = acc[:, :256] / acc[:, 256] ----
        den_sb = fin_pool.tile([SEGS_PER_CORE, 1], _F32, tag="den")
        nc.vector.tensor_scalar_add(den_sb[:], acc_ps[:, D:D + 1], 1e-30)
        rec_sb = fin_pool.tile([SEGS_PER_CORE, 1], _F32, tag="rec")
        nc.vector.reciprocal(rec_sb[:], den_sb[:])
        out_sb = fin_pool.tile([SEGS_PER_CORE, D], _F32, tag="out")
        nc.vector.tensor_scalar_mul(out_sb[:], acc_ps[:, 0:D], rec_sb[:])
        nc.sync.dma_start(out_d[:], out_sb[:])

    return nc


def _prepare_inputs(x, W1, b1, W2, b2, batch):
    x = np.ascontiguousarray(np.asarray(x, np.float32))
    batch = np.asarray(batch).astype(np.int64)
    # core k owns segments [64k, 64(k+1)); sorted batch -> contiguous ranges
    bounds = np.searchsorted(batch, np.arange(0, NUM_GRAPHS + 1, SEGS_PER_CORE))
    counts = np.diff(bounds)
    nmax = int(np.max(counts))
    n_chunks = max(1, -(-nmax // CHUNK_N))
    nmax_pad = n_chunks * CHUNK_N
    NC = n_chunks
    bf16 = ml_dtypes.bfloat16

    w1_h = np.ascontiguousarray(
        np.asarray(W1, np.float32).reshape(2, P, H).transpose(1, 0, 2)).astype(bf16)
    w2_h = np.asarray(W2, np.float32).reshape(H, 1).astype(bf16)
    b1_h = np.asarray(b1, np.float32).reshape(H, 1)
    iota_h = np.ascontiguousarray(np.broadcast_to(
        np.tile(np.arange(SEGS_PER_CORE, dtype=np.float32), CHUNK_T),
        (P, CHUNK_T * SEGS_PER_CORE))).astype(bf16)

    in_maps = []
    for k in range(N_CORES):
        lo, hi = int(bounds[k]), int(bounds[k + 1])
        cnt = hi - lo
        xcb = np.zeros((nmax_pad, D), bf16)
        xcb[:cnt] = x[lo:hi].astype(bf16)
        # xt[c, p, hf, n] = x[c*2048 + n, 128*hf + p]  (channel-partitioned)
        xt = np.ascontiguousarray(
            xcb.reshape(NC, CHUNK_N, 2, P).transpose(0, 3, 2, 1))
        # xb[c, p, t, ch] = x[c*2048 + 128*t + p, ch]; xb[..., 256] = 1
        xb = np.empty((NC, P, CHUNK_T, D + 1), bf16)
        xb[..., :D] = xcb.reshape(NC, CHUNK_T, P, D).transpose(0, 2, 1, 3)
        xb[..., D] = 1.0
        # bt[p, c, t] = local segment of node c*2048 + 128*t + p  (-1 = pad)
        lb = np.full((nmax_pad,), -1.0, np.float32)
        lb[:cnt] = (batch[lo:hi] - k * SEGS_PER_CORE).astype(np.float32)
        bt = np.ascontiguousarray(
            lb.reshape(NC, CHUNK_T, P).transpose(2, 0, 1)).astype(bf16)
        in_maps.append({
            "xt": xt,
            "xb": xb,
            "bt": bt,
            "iota": iota_h,
            "w1": w1_h,
            "w2": w2_h,
            "b1": b1_h,
        })
    return in_maps, n_chunks


def run(x, W1, b1, W2, b2, batch, trace=False, trace_kwargs=None):
    in_maps, n_chunks = _prepare_inputs(x, W1, b1, W2, b2, batch)
    nc = _build_program(n_chunks, float(np.asarray(b2).reshape(-1)[0]))
    nc.finalize()
    res = run_bass_kernel_spmd(nc, in_maps, list(range(N_CORES)),
                               trace=trace, **(trace_kwargs or {}))
    out = np.concatenate([np.asarray(res.results[k]["out_g"], np.float32)
                          for k in range(N_CORES)], axis=0)
    return out, res


def kernel(x, W1, b1, W2, b2, batch):
    out, _ = run(x, W1, b1, W2, b2, batch)
    return out


# revision 7
# speedup vs baseline: 1.1198x; 1.1172x over previous
"""AttentionPooling (segment softmax + weighted segment sum) on 8 trn2 cores.

Strategy: shard whole segments across cores (sorted batch -> contiguous node
ranges).  The host pre-casts x to bf16 and ships it twice: channel-partitioned
(xt, for the MLP matmuls) and node-partitioned with an appended ones-channel
(xb, for the weighted-sum matmul whose 257th output column is the softmax
denominator).  Total HBM traffic per core = 2 x 32 MB bf16 = the same 64 MB a
single fp32 copy would cost, with zero on-device casts/transposes/bounces.
Per 2048-node chunk: 8 MLP matmuls (512-col), 16 1-col score matmuls, tanh+exp
on ACT, bf16 compare/mult on DVE for we = onehot(segment)*e, 16 wsum matmuls
accumulating (64,257) in PSUM.  Softmax max-subtraction is skipped:
|s| <= ||W2||_1 + |b2| ~ 28, exp stays in fp32/bf16 range.
"""

from contextlib import ExitStack

import numpy as np
import ml_dtypes

import concourse.bass as bass
import concourse.bacc as bacc
import concourse.tile as tile
from concourse import mybir
from concourse.bass_utils import run_bass_kernel_spmd

N_CORES = 8
NUM_GRAPHS = 512
SEGS_PER_CORE = NUM_GRAPHS // N_CORES  # 64
D = 256          # in channels
H = 128          # hidden
P = 128          # partitions
TILE_N = 128     # nodes per score tile
CHUNK_T = 16     # tiles per chunk
CHUNK_N = TILE_N * CHUNK_T  # 2048 nodes per chunk

_BF16 = mybir.dt.bfloat16
_F32 = mybir.dt.float32


def _build_program(n_chunks: int, b2_val: float):
    nc = bacc.Bacc()
    NC = n_chunks

    xt_d = nc.declare_dram_parameter("xt", [NC, P, 2, CHUNK_N], _BF16, isOutput=False)
    xb_d = nc.declare_dram_parameter("xb", [NC, P, CHUNK_T, D + 1], _BF16, isOutput=False)
    bt_d = nc.declare_dram_parameter("bt", [P, NC, CHUNK_T], _BF16, isOutput=False)
    io_d = nc.declare_dram_parameter("iota", [P, CHUNK_T * SEGS_PER_CORE], _BF16, isOutput=False)
    w1_d = nc.declare_dram_parameter("w1", [P, 2, H], _BF16, isOutput=False)
    w2_d = nc.declare_dram_parameter("w2", [P, 1], _BF16, isOutput=False)
    b1_d = nc.declare_dram_parameter("b1", [P, 1], _F32, isOutput=False)
    out_d = nc.declare_dram_parameter("out_g", [SEGS_PER_CORE, D], _F32, isOutput=True)

    with tile.TileContext(nc) as tc, ExitStack() as ctx:
        const_pool = ctx.enter_context(tc.tile_pool(name="consts", bufs=1))
        xt_pool = ctx.enter_context(tc.tile_pool(name="xt", bufs=4))
        xb_pool = ctx.enter_context(tc.tile_pool(name="xb", bufs=4))
        h_pool = ctx.enter_context(tc.tile_pool(name="h", bufs=2))
        we_pool = ctx.enter_context(tc.tile_pool(name="we", bufs=3))
        e_pool = ctx.enter_context(tc.tile_pool(name="e", bufs=2))
        fin_pool = ctx.enter_context(tc.tile_pool(name="fin", bufs=1))
        psum_h = ctx.enter_context(
            tc.tile_pool(name="psum_h", bufs=3, space=bass.MemorySpace.PSUM))
        psum_s = ctx.enter_context(
            tc.tile_pool(name="psum_s", bufs=2, space=bass.MemorySpace.PSUM))
        psum_acc = ctx.enter_context(
            tc.tile_pool(name="psum_acc", bufs=1, space=bass.MemorySpace.PSUM))

        # ---- constants / weights ----
        w1_sb = const_pool.tile([P, 2, H], _BF16, tag="w1")
        nc.scalar.dma_start(w1_sb[:], w1_d[:])
        w2_sb = const_pool.tile([P, 1], _BF16, tag="w2")
        nc.scalar.dma_start(w2_sb[:], w2_d[:])
        b1_sb = const_pool.tile([P, 1], _F32, tag="b1")
        nc.scalar.dma_start(b1_sb[:], b1_d[:])
        bt_sb = const_pool.tile([P, NC, CHUNK_T], _BF16, tag="bt")
        nc.scalar.dma_start(bt_sb[:], bt_d[:])
        io_sb = const_pool.tile([P, CHUNK_T * SEGS_PER_CORE], _BF16, tag="iota")
        nc.scalar.dma_start(io_sb[:], io_d[:])
        io_v = io_sb[:].rearrange("p (t g) -> p t g", g=SEGS_PER_CORE)

        acc_ps = psum_acc.tile([SEGS_PER_CORE, D + 1], _F32, tag="acc")

        # software pipeline over global slices k = 4c + s.  At step k we emit:
        #   1. the MLP matmul pair + tanh for slice k,
        #   2. the score matmuls for slice k-1 (so the PE never waits on the
        #      tanh latency of its own slice),
        #   3. one 4-tile wsum group, delayed 5 slices behind its chunk's
        #      scores (so `we` is long since ready).
        state = {}

        def mlp_slice(k):
            c, s = divmod(k, 4)
            if s == 0:
                xt = xt_pool.tile([P, 2, CHUNK_N], _BF16, tag="xt")
                nc.sync.dma_start(xt[:], xt_d[c])
                xb = xb_pool.tile([P, CHUNK_T, D + 1], _BF16, tag="xb")
                nc.gpsimd.dma_start(xb[:, 0:CHUNK_T // 2, :],
                                    xb_d[c, :, 0:CHUNK_T // 2, :])
                nc.scalar.dma_start(xb[:, CHUNK_T // 2:, :],
                                    xb_d[c, :, CHUNK_T // 2:, :])
                h_bf = h_pool.tile([P, CHUNK_N], _BF16, tag="h")
                ps_s = psum_s.tile([P, CHUNK_T], _F32, tag="ps_s")
                state[c] = {"xt": xt, "xb": xb, "h": h_bf, "ps": ps_s}
            st = state[c]
            ph = psum_h.tile([P, 512], _F32, tag="ph")
            sl = slice(s * 512, (s + 1) * 512)
            nc.tensor.matmul(ph[:], w1_sb[:, 0, :], st["xt"][:, 0, sl],
                             start=True, stop=False)
            nc.tensor.matmul(ph[:], w1_sb[:, 1, :], st["xt"][:, 1, sl],
                             start=False, stop=True)
            nc.scalar.activation(st["h"][:, sl], ph[:],
                                 mybir.ActivationFunctionType.Tanh,
                                 bias=b1_sb[:])

        def score_slice(k):
            c, s = divmod(k, 4)
            st = state[c]
            for t in range(4 * s, 4 * s + 4):
                nc.tensor.matmul(st["ps"][:, t:t + 1],
                                 st["h"][:, t * TILE_N:(t + 1) * TILE_N],
                                 w2_sb[:], start=True, stop=True)
            if s == 3:
                e_col = e_pool.tile([P, CHUNK_T], _BF16, tag="e")
                nc.scalar.activation(e_col[:], st["ps"][:],
                                     mybir.ActivationFunctionType.Exp,
                                     bias=float(b2_val))
                # we[p, t, g] = (bt[p, t] == g) * e[p, t]   (all bf16)
                we = we_pool.tile([P, CHUNK_T, SEGS_PER_CORE], _BF16, tag="we")
                nc.vector.tensor_tensor(
                    we[:], io_v,
                    bt_sb[:, c, :].unsqueeze(2).broadcast_to(
                        [P, CHUNK_T, SEGS_PER_CORE]),
                    mybir.AluOpType.is_equal)
                nc.vector.tensor_tensor(
                    we[:], we[:],
                    e_col[:].unsqueeze(2).broadcast_to(
                        [P, CHUNK_T, SEGS_PER_CORE]),
                    mybir.AluOpType.mult)
                st["we"] = we

        def wsum_group(j):
            c, grp = divmod(j, 4)
            st = state[c]
            for t in range(4 * grp, 4 * grp + 4):
                nc.tensor.matmul(acc_ps[:], st["we"][:, t, :],
                                 st["xb"][:, t, :],
                                 start=(c == 0 and t == 0),
                                 stop=(c == n_chunks - 1 and t == CHUNK_T - 1),
                                 skip_group_check=True)
            if grp == 3:
                del state[c]

        n_slices = 4 * n_chunks
        for k in range(n_slices + 14):
            if k < n_slices:
                mlp_slice(k)
            if 1 <= k <= n_slices:
                score_slice(k - 1)
            if 9 <= k < n_slices + 9:
                wsum_group(k - 9)

        # ---- epilogue: out = acc[:, :256] / acc[:, 256] ----
        den_sb = fin_pool.tile([SEGS_PER_CORE, 1], _F32, tag="den")
        nc.vector.tensor_scalar_add(den_sb[:], acc_ps[:, D:D + 1], 1e-30)
        rec_sb = fin_pool.tile([SEGS_PER_CORE, 1], _F32, tag="rec")
        nc.vector.reciprocal(rec_sb[:], den_sb[:])
        out_sb = fin_pool.tile([SEGS_PER_CORE, D], _F32, tag="out")
        nc.vector.tensor_scalar_mul(out_sb[:], acc_ps[:, 0:D], rec_sb[:])
        nc.sync.dma_start(out_d[:], out_sb[:])

    return nc


def _prepare_inputs(x, W1, b1, W2, b2, batch):
    x = np.ascontiguousarray(np.asarray(x, np.float32))
    batch = np.asarray(batch).astype(np.int64)
    # core k owns segments [64k, 64(k+1)); sorted batch -> contiguous ranges
    bounds = np.searchsorted(batch, np.arange(0, NUM_GRAPHS + 1, SEGS_PER_CORE))
    counts = np.diff(bounds)
    nmax = int(np.max(counts))
    n_chunks = max(1, -(-nmax // CHUNK_N))
    nmax_pad = n_chunks * CHUNK_N
    NC = n_chunks
    bf16 = ml_dtypes.bfloat16

    w1_h = np.ascontiguousarray(
        np.asarray(W1, np.float32).reshape(2, P, H).transpose(1, 0, 2)).astype(bf16)
    w2_h = np.asarray(W2, np.float32).reshape(H, 1).astype(bf16)
    b1_h = np.asarray(b1, np.float32).reshape(H, 1)
    iota_h = np.ascontiguousarray(np.broadcast_to(
        np.tile(np.arange(SEGS_PER_CORE, dtype=np.float32), CHUNK_T),
        (P, CHUNK_T * SEGS_PER_CORE))).astype(bf16)

    in_maps = []
    for k in range(N_CORES):
        lo, hi = int(bounds[k]), int(bounds[k + 1])
        cnt = hi - lo
        xcb = np.zeros((nmax_pad, D), bf16)
        xcb[:cnt] = x[lo:hi].astype(bf16)
        # xt[c, p, hf, n] = x[c*2048 + n, 128*hf + p]  (channel-partitioned)
        xt = np.ascontiguousarray(
            xcb.reshape(NC, CHUNK_N, 2, P).transpose(0, 3, 2, 1))
        # xb[c, p, t, ch] = x[c*2048 + 128*t + p, ch]; xb[..., 256] = 1
        xb = np.empty((NC, P, CHUNK_T, D + 1), bf16)
        xb[..., :D] = xcb.reshape(NC, CHUNK_T, P, D).transpose(0, 2, 1, 3)
        xb[..., D] = 1.0
        # bt[p, c, t] = local segment of node c*2048 + 128*t + p  (-1 = pad)
        lb = np.full((nmax_pad,), -1.0, np.float32)
        lb[:cnt] = (batch[lo:hi] - k * SEGS_PER_CORE).astype(np.float32)
        bt = np.ascontiguousarray(
            lb.reshape(NC, CHUNK_T, P).transpose(2, 0, 1)).astype(bf16)
        in_maps.append({
            "xt": xt,
            "xb": xb,
            "bt": bt,
            "iota": iota_h,
            "w1": w1_h,
            "w2": w2_h,
            "b1": b1_h,
        })
    return in_maps, n_chunks


def run(x, W1, b1, W2, b2, batch, trace=False, trace_kwargs=None):
    in_maps, n_chunks = _prepare_inputs(x, W1, b1, W2, b2, batch)
    nc = _build_program(n_chunks, float(np.asarray(b2).reshape(-1)[0]))
    nc.finalize()
    res = run_bass_kernel_spmd(nc, in_maps, list(range(N_CORES)),
                               trace=trace, **(trace_kwargs or {}))
    out = np.concatenate([np.asarray(res.results[k]["out_g"], np.float32)
                          for k in range(N_CORES)], axis=0)
    return out, res


def kernel(x, W1, b1, W2, b2, batch):
    out, _ = run(x, W1, b1, W2, b2, batch)
    return out
